# revision 1
# baseline (speedup 1.0000x reference)
import sys

sys.path.insert(0, "/opt/trn_rl_repo")

import numpy as np

import concourse.bacc as bacc
import concourse.bass as bass
import concourse.mybir as mybir
import concourse.tile as tile
from concourse.bass_utils import run_bass_kernel_spmd

# Problem shapes (hardcoded per contract)
B = 4
NQ = 2048
NR = 16384
D = 64
K = 16

NCORES = 8
QPC = NQ // 2          # queries per core (each batch split across 2 cores)
NCHUNK = QPC // 128    # query chunks of 128 per core
MMN = 512              # matmul free dim (one PSUM bank of fp32)
GRP = 1024             # candidate block width (2 PSUM banks); top-8 per group
NGRP = NR // GRP       # 16 groups
NCAND = NGRP * 8       # 128 candidates per row

_prog_cache = {}


def _build_program(reps: int = 1):
    if reps in _prog_cache:
        return _prog_cache[reps]

    f32 = mybir.dt.float32
    u32 = mybir.dt.uint32

    nc = bacc.Bacc("TRN2", target_bir_lowering=False, debug=False, num_devices=NCORES)

    # lhsT rows 0..63 = 2*q^T, row 64 = 1.0, row 65 = q2  -> psum = 2qr - r2 - q2 = -d2
    lhs_d = nc.dram_tensor("lhs", [66, QPC], f32, kind="ExternalInput")
    rhs_d = nc.dram_tensor("rhs", [66, NR], f32, kind="ExternalInput")

    outD_d = nc.dram_tensor("outD", [QPC, K], f32, kind="ExternalOutput")
    outP_d = nc.dram_tensor("outP", [QPC, K], u32, kind="ExternalOutput")
    outCI_d = nc.dram_tensor("outCI", [QPC, NCAND], u32, kind="ExternalOutput")

    with tile.TileContext(nc) as tc:
        with (
            tc.tile_pool(name="consts", bufs=1) as cpool,
            tc.tile_pool(name="psum", bufs=8, space="PSUM") as ppool,
            tc.tile_pool(name="stage", bufs=6) as spool,
            tc.tile_pool(name="cands", bufs=3) as candpool,
            tc.tile_pool(name="merge", bufs=2) as mpool,
        ):
            lhs_t = cpool.tile([66, QPC], f32)
            nc.sync.dma_start(lhs_t[:], lhs_d.ap())
            rhs_t = cpool.tile([66, NR], f32)
            nc.sync.dma_start(rhs_t[:], rhs_d.ap())

            for rep in range(reps):
              for c in range(NCHUNK):
                cands_v = candpool.tile([128, NCAND], f32, tag="cv")
                cands_i = candpool.tile([128, NCAND], u32, tag="ci")
                for g in range(NGRP):
                    st = spool.tile([128, GRP], f32, tag="st")
                    for h in range(GRP // MMN):
                        ps = ppool.tile([128, MMN], f32, tag="ps")
                        nc.tensor.matmul(
                            ps[:],
                            lhs_t[:, c * 128:(c + 1) * 128],
                            rhs_t[:, g * GRP + h * MMN:g * GRP + (h + 1) * MMN],
                            start=True,
                            stop=True,
                        )
                        nc.scalar.copy(st[:, h * MMN:(h + 1) * MMN], ps[:])
                    s = g * 8
                    nc.vector.max(cands_v[:, s:s + 8], st[:])
                    nc.vector.max_index(cands_i[:, s:s + 8], cands_v[:, s:s + 8], st[:])

                # merge candidates -> top-16 (values + candidate slots)
                v16 = mpool.tile([128, K], f32, tag="v16")
                p16 = mpool.tile([128, K], u32, tag="p16")
                mr = mpool.tile([128, NCAND], f32, tag="mr")
                nc.vector.max(v16[:, 0:8], cands_v[:])
                nc.vector.max_index(p16[:, 0:8], v16[:, 0:8], cands_v[:])
                nc.vector.match_replace(mr[:], v16[:, 0:8], cands_v[:], -1e30)
                nc.vector.max(v16[:, 8:16], mr[:])
                nc.vector.max_index(p16[:, 8:16], v16[:, 8:16], mr[:])

                # D = sqrt(relu(-v16))
                dsq = mpool.tile([128, K], f32, tag="dsq")
                d16 = mpool.tile([128, K], f32, tag="d16")
                nc.scalar.activation(
                    dsq[:], v16[:], mybir.ActivationFunctionType.Relu, scale=-1.0
                )
                nc.scalar.activation(d16[:], dsq[:], mybir.ActivationFunctionType.Sqrt)

                r0, r1 = c * 128, (c + 1) * 128
                nc.sync.dma_start(outD_d.ap()[r0:r1, :], d16[:])
                nc.sync.dma_start(outP_d.ap()[r0:r1, :], p16[:])
                nc.sync.dma_start(outCI_d.ap()[r0:r1, :], cands_i[:])

    nc.compile()
    _prog_cache[reps] = nc
    return nc


def kernel(ref: np.ndarray, query: np.ndarray):
    ref = np.asarray(ref, dtype=np.float32)
    query = np.asarray(query, dtype=np.float32)

    # host-side operand prep (layout + norms)
    r2 = np.sum(ref * ref, axis=-1)                      # [B, NR]
    q2 = np.sum(query * query, axis=-1)                  # [B, NQ]
    refT = np.ascontiguousarray(ref.transpose(0, 2, 1))  # [B, D, NR]
    qT = np.ascontiguousarray(query.transpose(0, 2, 1))  # [B, D, NQ]

    nc = _build_program()

    in_maps = []
    for core in range(NCORES):
        b, h = core // 2, core % 2
        lhs = np.empty((66, QPC), dtype=np.float32)
        lhs[0:D, :] = 2.0 * qT[b][:, h * QPC:(h + 1) * QPC]
        lhs[D, :] = 1.0
        lhs[D + 1, :] = q2[b, h * QPC:(h + 1) * QPC]
        rhs = np.empty((66, NR), dtype=np.float32)
        rhs[0:D, :] = refT[b]
        rhs[D, :] = -r2[b]
        rhs[D + 1, :] = -1.0
        in_maps.append({"lhs": lhs, "rhs": rhs})

    res = run_bass_kernel_spmd(nc, in_maps, core_ids=list(range(NCORES)))

    Dout = np.empty((B, NQ, K), dtype=np.float32)
    Iout = np.empty((B, NQ, K), dtype=np.int64)
    rows = np.arange(QPC)[:, None]
    for core in range(NCORES):
        b, h = core // 2, core % 2
        r = res.results[core]
        d16 = r["outD"]                      # [QPC, K] f32
        p16 = r["outP"].astype(np.int64)     # [QPC, K] candidate slots
        ci = r["outCI"].astype(np.int64)     # [QPC, NCAND] local idx in group
        gi = ci[rows, p16] + GRP * (p16 >> 3)
        Dout[b, h * QPC:(h + 1) * QPC] = d16
        Iout[b, h * QPC:(h + 1) * QPC] = gi
    return (Dout, Iout)



# revision 7
# speedup vs baseline: 1.7510x; 1.7510x over previous
import sys

sys.path.insert(0, "/opt/trn_rl_repo")

import numpy as np

import concourse.bacc as bacc
import concourse.bass as bass
import concourse.mybir as mybir
import concourse.tile as tile
from concourse.bass_utils import run_bass_kernel_spmd

# Problem shapes (hardcoded per contract)
B = 4
NQ = 2048
NR = 16384
D = 64
K = 16

NCORES = 8
QPC = NQ // 2          # queries per core (each batch split across 2 cores)
NCHUNK = QPC // 128    # query chunks of 128 per core
GRP = 1024             # candidate group width (2 PSUM banks); top-8 per group
NGRP = NR // GRP       # 16 groups
NCAND = NGRP * 8       # 128 candidates per row

# Packed-key selection (one DVE Max8 sweep replaces Max+MaxIndex):
#   psum = 2^15*(C - d2)                  (scaled matmul, fp32)
#   += 2^33 ; -= 2^33 (1-row matmuls)     (fp32 accumulate rounds to m*1024)
#   key  = psum + (1023-j)                (exact ints < 2^24; j = idx in group)
# Max key == min d2 quantized to 1/32, ties -> lower idx. Host merges the
# top-32 of 128 candidate keys per row and reranks with exact float64 d2.
#
# The icon (1023-j) add is routed per group to spread engine load:
#   'pe'  : 4th accumulating 1-row matmul; Max8 reads the key from PSUM
#   'dve' : DVE STT key = (icon + 0) + psum -> SBUF
#   'pool': Act evicts v = Copy(psum - 2^33) (skips PE demagic), then
#           Pool TT key = v + icon
CCONST = 448.0
MAGIC = 2.0 ** 33
T32 = 32               # candidates reranked on host

ROUTES = [
    "pe", "pool", "dve", "pool", "pe", "pool", "pe", "pool",
    "pe", "pool", "pool", "pe", "pool", "pe", "pool", "pool",
]

_prog_cache = {}


def _build_program(reps: int = 1):
    if reps in _prog_cache:
        return _prog_cache[reps]

    f32 = mybir.dt.float32
    f32r = mybir.dt.float32r
    add = mybir.AluOpType.add
    Copy = mybir.ActivationFunctionType.Copy

    nc = bacc.Bacc("TRN2", target_bir_lowering=False, debug=False, num_devices=NCORES)

    # lhsT rows 0..63 = 2^16*q^T, row 64 = 2^15, row 65 = 2^15*q2
    # rhs  rows 0..63 = r^T,      row 64 = C-r2, row 65 = -1
    #   -> psum = 2^15 * (2q.r + C - r2 - q2) = 2^15 * (C - d2)
    lhs_d = nc.dram_tensor("lhs", [66, QPC], f32r, kind="ExternalInput")
    rhs_d = nc.dram_tensor("rhs", [66, NR], f32r, kind="ExternalInput")
    icon_d = nc.dram_tensor("icon", [128, GRP], f32, kind="ExternalInput")
    mlhs_d = nc.dram_tensor("mlhs", [1, 128], f32r, kind="ExternalInput")
    mrhs_d = nc.dram_tensor("mrhs", [1, 512], f32r, kind="ExternalInput")
    nrhs_d = nc.dram_tensor("nrhs", [1, 512], f32r, kind="ExternalInput")
    ic0_d = nc.dram_tensor("ic0", [1, 512], f32r, kind="ExternalInput")
    ic1_d = nc.dram_tensor("ic1", [1, 512], f32r, kind="ExternalInput")

    outK_d = nc.dram_tensor("outK", [QPC, NCAND], f32, kind="ExternalOutput")

    NRHS = 4  # rhs loaded as 4 tiles so compute starts before the full load
    RW = NR // NRHS

    with tile.TileContext(nc) as tc:
        with (
            tc.tile_pool(name="consts", bufs=1) as cpool,
            tc.tile_pool(name="psum", bufs=4, space="PSUM") as ppool,
            tc.tile_pool(name="keys", bufs=4) as kpool,
            tc.tile_pool(name="vtmp", bufs=3) as vpool,
            tc.tile_pool(name="cands", bufs=2) as candpool,
        ):
            lhs_t = cpool.tile([66, QPC], f32r, tag="lhs")
            nc.sync.dma_start(lhs_t[:], lhs_d.ap())
            icon_t = cpool.tile([128, GRP], f32, tag="icon")
            nc.sync.dma_start(icon_t[:], icon_d.ap())
            mlhs_t = cpool.tile([1, 128], f32r, tag="mlhs")
            nc.sync.dma_start(mlhs_t[:], mlhs_d.ap())
            mrhs_t = cpool.tile([1, 512], f32r, tag="mrhs")
            nc.sync.dma_start(mrhs_t[:], mrhs_d.ap())
            nrhs_t = cpool.tile([1, 512], f32r, tag="nrhs")
            nc.sync.dma_start(nrhs_t[:], nrhs_d.ap())
            ic_t = []
            for nm, d in (("ic0", ic0_d), ("ic1", ic1_d)):
                t = cpool.tile([1, 512], f32r, tag=nm)
                nc.sync.dma_start(t[:], d.ap())
                ic_t.append(t)
            rhs_t = []
            for i in range(NRHS):
                t = cpool.tile([66, RW], f32r, tag=f"rhs{i}")
                nc.sync.dma_start(t[:], rhs_d.ap()[:, i * RW:(i + 1) * RW])
                rhs_t.append(t)

            for rep in range(reps):
              for c in range(NCHUNK):
                cands = candpool.tile([128, NCAND], f32, tag="cv")
                for g in range(NGRP):
                    route = ROUTES[g]
                    rt = rhs_t[(g * GRP) // RW]
                    roff = (g * GRP) % RW
                    ps = ppool.tile([128, GRP], f32, tag="ps")
                    for h in range(2):
                        bank = ps[:, h * 512:(h + 1) * 512]
                        nc.tensor.matmul(
                            bank,
                            lhs_t[:, c * 128:(c + 1) * 128],
                            rt[:, roff + h * 512:roff + (h + 1) * 512],
                            start=True, stop=False,
                        )
                        # +2^33: rounds psum to the 1024 grid
                        last = route == "pool"
                        nc.tensor.matmul(bank, mlhs_t[:], mrhs_t[:],
                                         start=False, stop=last)
                        if not last:
                            # -2^33: psum = m*1024 exactly
                            nc.tensor.matmul(bank, mlhs_t[:], nrhs_t[:],
                                             start=False, stop=route != "pe")
                            if route == "pe":
                                # + (1023-j): key complete in PSUM
                                nc.tensor.matmul(bank, mlhs_t[:], ic_t[h][:],
                                                 start=False, stop=True)

                    s = g * 8
                    if route == "pe":
                        nc.vector.max(cands[:, s:s + 8], ps[:])
                        continue
                    key = kpool.tile([128, GRP], f32, tag="key")
                    if route == "dve":
                        nc.vector.scalar_tensor_tensor(
                            key[:], icon_t[:], 0.0, ps[:], op0=add, op1=add
                        )
                    else:  # pool
                        v = vpool.tile([128, GRP], f32, tag="v")
                        nc.scalar.activation(v[:], ps[:], Copy, bias=-MAGIC)
                        nc.gpsimd.tensor_tensor(
                            out=key[:], in0=v[:], in1=icon_t[:], op=add
                        )
                    nc.vector.max(cands[:, s:s + 8], key[:])

                r0, r1 = c * 128, (c + 1) * 128
                nc.sync.dma_start(outK_d.ap()[r0:r1, :], cands[:])

    nc.compile()
    _prog_cache[reps] = nc
    return nc


def kernel(ref: np.ndarray, query: np.ndarray):
    ref = np.asarray(ref, dtype=np.float32)
    query = np.asarray(query, dtype=np.float32)

    # host-side operand prep (layout + norms)
    r2 = np.sum(ref * ref, axis=-1)                      # [B, NR]
    q2 = np.sum(query * query, axis=-1)                  # [B, NQ]
    refT = np.ascontiguousarray(ref.transpose(0, 2, 1))  # [B, D, NR]
    qT = np.ascontiguousarray(query.transpose(0, 2, 1))  # [B, D, NQ]

    nc = _build_program()

    iconrow = float(GRP - 1) - np.arange(GRP, dtype=np.float32)
    icon = np.broadcast_to(iconrow[None, :], (128, GRP)).copy()
    mlhs = np.ones((1, 128), dtype=np.float32)
    mrhs = np.full((1, 512), MAGIC, dtype=np.float32)
    nrhs = np.full((1, 512), -MAGIC, dtype=np.float32)
    ic0 = iconrow[None, :512].copy()
    ic1 = iconrow[None, 512:].copy()

    in_maps = []
    for core in range(NCORES):
        b, h = core // 2, core % 2
        lhs = np.empty((66, QPC), dtype=np.float32)
        lhs[0:D, :] = 65536.0 * qT[b][:, h * QPC:(h + 1) * QPC]
        lhs[D, :] = 32768.0
        lhs[D + 1, :] = 32768.0 * q2[b, h * QPC:(h + 1) * QPC]
        rhs = np.empty((66, NR), dtype=np.float32)
        rhs[0:D, :] = refT[b]
        rhs[D, :] = CCONST - r2[b]
        rhs[D + 1, :] = -1.0
        in_maps.append(
            {"lhs": lhs, "rhs": rhs, "icon": icon, "mlhs": mlhs,
             "mrhs": mrhs, "nrhs": nrhs, "ic0": ic0, "ic1": ic1}
        )

    res = run_bass_kernel_spmd(nc, in_maps, core_ids=list(range(NCORES)))

    Dout = np.empty((B, NQ, K), dtype=np.float32)
    Iout = np.empty((B, NQ, K), dtype=np.int64)
    for core in range(NCORES):
        b, h = core // 2, core % 2
        keys = res.results[core]["outK"]                  # [QPC, NCAND] f32
        kint = keys.astype(np.int64)                      # exact ints < 2^24
        top = np.argpartition(-keys, T32, axis=1)[:, :T32]
        kv = np.take_along_axis(kint, top, axis=1)
        j = (GRP - 1) - (kv & (GRP - 1))
        idx = (top >> 3) * GRP + j                        # [QPC, T32]

        # exact float64 rerank of the 32 candidates
        qb = query[b, h * QPC:(h + 1) * QPC].astype(np.float64)   # [QPC, D]
        rsel = ref[b].astype(np.float64)[idx]                     # [QPC, T32, D]
        diff = qb[:, None, :] - rsel
        d2 = np.einsum("qtd,qtd->qt", diff, diff)
        order = np.lexsort((idx, d2), axis=1)[:, :K]
        selI = np.take_along_axis(idx, order, axis=1)
        selD = np.sqrt(np.take_along_axis(d2, order, axis=1))

        Dout[b, h * QPC:(h + 1) * QPC] = selD.astype(np.float32)
        Iout[b, h * QPC:(h + 1) * QPC] = selI
    return (Dout, Iout)


# revision 10
# speedup vs baseline: 1.7598x; 1.0050x over previous
import sys

sys.path.insert(0, "/opt/trn_rl_repo")

import numpy as np

import concourse.bacc as bacc
import concourse.bass as bass
import concourse.mybir as mybir
import concourse.tile as tile
from concourse.bass_utils import run_bass_kernel_spmd

# Problem shapes (hardcoded per contract)
B = 4
NQ = 2048
NR = 16384
D = 64
K = 16

NCORES = 8
QPC = NQ // 2          # queries per core (each batch split across 2 cores)
NCHUNK = QPC // 128    # query chunks of 128 per core
GRP = 1024             # candidate group width (2 PSUM banks); top-8 per group
NGRP = NR // GRP       # 16 groups
NCAND = NGRP * 8       # 128 candidates per row

# Packed-key selection (one DVE Max8 sweep replaces Max+MaxIndex):
#   psum = 2^15*(C - d2)                  (scaled matmul, fp32)
#   += 2^33 ; -= 2^33 (1-row matmuls)     (fp32 accumulate rounds to m*1024)
#   key  = psum + (1023-j)                (exact ints < 2^24; j = idx in group)
# Max key == min d2 quantized to 1/32, ties -> lower idx. Host merges the
# top-32 of 128 candidate keys per row and reranks with exact float64 d2.
#
# The icon (1023-j) add is routed per group to spread engine load:
#   'pe'  : 4th accumulating 1-row matmul; Max8 reads the key from PSUM
#   'dve' : DVE STT key = (icon + 0) + psum -> SBUF
#   'pool': Act evicts v = Copy(psum - 2^33) (skips PE demagic), then
#           Pool TT key = v + icon
CCONST = 448.0
MAGIC = 2.0 ** 33
T32 = 32               # candidates reranked on host

ROUTES = [
    "pe", "pool", "dve", "pool", "pe", "pool", "pe", "pool",
    "pe", "pool", "pool", "pe", "pool", "pe", "pool", "pool",
]

_prog_cache = {}


def _build_program(reps: int = 1):
    if reps in _prog_cache:
        return _prog_cache[reps]

    f32 = mybir.dt.float32
    f32r = mybir.dt.float32r
    add = mybir.AluOpType.add
    Copy = mybir.ActivationFunctionType.Copy

    nc = bacc.Bacc("TRN2", target_bir_lowering=False, debug=False, num_devices=NCORES)

    # lhsT rows 0..63 = 2^16*q^T, row 64 = 2^15, row 65 = 2^15*q2
    # rhs  rows 0..63 = r^T,      row 64 = C-r2, row 65 = -1
    #   -> psum = 2^15 * (2q.r + C - r2 - q2) = 2^15 * (C - d2)
    lhs_d = nc.dram_tensor("lhs", [66, QPC], f32r, kind="ExternalInput")
    rhs_d = nc.dram_tensor("rhs", [66, NR], f32r, kind="ExternalInput")
    icon_d = nc.dram_tensor("icon", [128, GRP], f32, kind="ExternalInput")
    mlhs_d = nc.dram_tensor("mlhs", [1, 128], f32r, kind="ExternalInput")
    mrhs_d = nc.dram_tensor("mrhs", [1, 512], f32r, kind="ExternalInput")
    nrhs_d = nc.dram_tensor("nrhs", [1, 512], f32r, kind="ExternalInput")
    ic0_d = nc.dram_tensor("ic0", [1, 512], f32r, kind="ExternalInput")
    ic1_d = nc.dram_tensor("ic1", [1, 512], f32r, kind="ExternalInput")

    outK_d = nc.dram_tensor("outK", [QPC, NCAND], f32, kind="ExternalOutput")

    NRHS = 4  # rhs loaded as 4 tiles so compute starts before the full load
    RW = NR // NRHS

    with tile.TileContext(nc) as tc:
        with (
            tc.tile_pool(name="consts", bufs=1) as cpool,
            tc.tile_pool(name="psum", bufs=4, space="PSUM") as ppool,
            tc.tile_pool(name="keys", bufs=6) as kpool,
            tc.tile_pool(name="vtmp", bufs=4) as vpool,
            tc.tile_pool(name="cands", bufs=3) as candpool,
        ):
            mlhs_t = cpool.tile([1, 128], f32r, tag="mlhs")
            nc.sync.dma_start(mlhs_t[:], mlhs_d.ap())
            mrhs_t = cpool.tile([1, 512], f32r, tag="mrhs")
            nc.sync.dma_start(mrhs_t[:], mrhs_d.ap())
            nrhs_t = cpool.tile([1, 512], f32r, tag="nrhs")
            nc.sync.dma_start(nrhs_t[:], nrhs_d.ap())
            ic_t = []
            for nm, d in (("ic0", ic0_d), ("ic1", ic1_d)):
                t = cpool.tile([1, 512], f32r, tag=nm)
                nc.sync.dma_start(t[:], d.ap())
                ic_t.append(t)
            lhs_t = cpool.tile([66, QPC], f32r, tag="lhs")
            nc.sync.dma_start(lhs_t[:], lhs_d.ap())
            rhs0_t = cpool.tile([66, RW], f32r, tag="rhs0")
            nc.sync.dma_start(rhs0_t[:], rhs_d.ap()[:, 0:RW])
            icon_t = cpool.tile([128, GRP], f32, tag="icon")
            nc.sync.dma_start(icon_t[:], icon_d.ap())
            rhs_t = [rhs0_t]
            for i in range(1, NRHS):
                t = cpool.tile([66, RW], f32r, tag=f"rhs{i}")
                nc.sync.dma_start(t[:], rhs_d.ap()[:, i * RW:(i + 1) * RW])
                rhs_t.append(t)

            for rep in range(reps):
              for c in range(NCHUNK):
                cands = candpool.tile([128, NCAND], f32, tag="cv")
                for g in range(NGRP):
                    route = ROUTES[g]
                    rt = rhs_t[(g * GRP) // RW]
                    roff = (g * GRP) % RW
                    ps = ppool.tile([128, GRP], f32, tag="ps")
                    for h in range(2):
                        bank = ps[:, h * 512:(h + 1) * 512]
                        nc.tensor.matmul(
                            bank,
                            lhs_t[:, c * 128:(c + 1) * 128],
                            rt[:, roff + h * 512:roff + (h + 1) * 512],
                            start=True, stop=False,
                        )
                        # +2^33: rounds psum to the 1024 grid
                        last = route == "pool"
                        nc.tensor.matmul(bank, mlhs_t[:], mrhs_t[:],
                                         start=False, stop=last)
                        if not last:
                            # -2^33: psum = m*1024 exactly
                            nc.tensor.matmul(bank, mlhs_t[:], nrhs_t[:],
                                             start=False, stop=route != "pe")
                            if route == "pe":
                                # + (1023-j): key complete in PSUM
                                nc.tensor.matmul(bank, mlhs_t[:], ic_t[h][:],
                                                 start=False, stop=True)

                    s = g * 8
                    if route == "pe":
                        nc.vector.max(cands[:, s:s + 8], ps[:])
                        continue
                    key = kpool.tile([128, GRP], f32, tag="key")
                    if route == "dve":
                        nc.vector.scalar_tensor_tensor(
                            key[:], icon_t[:], 0.0, ps[:], op0=add, op1=add
                        )
                    else:  # pool
                        v = vpool.tile([128, GRP], f32, tag="v")
                        nc.scalar.activation(v[:], ps[:], Copy, bias=-MAGIC)
                        nc.gpsimd.tensor_tensor(
                            out=key[:], in0=v[:], in1=icon_t[:], op=add
                        )
                    nc.vector.max(cands[:, s:s + 8], key[:])

                r0, r1 = c * 128, (c + 1) * 128
                nc.sync.dma_start(outK_d.ap()[r0:r1, :], cands[:])

    nc.compile()
    _prog_cache[reps] = nc
    return nc


def kernel(ref: np.ndarray, query: np.ndarray):
    ref = np.asarray(ref, dtype=np.float32)
    query = np.asarray(query, dtype=np.float32)

    # host-side operand prep (layout + norms)
    r2 = np.sum(ref * ref, axis=-1)                      # [B, NR]
    q2 = np.sum(query * query, axis=-1)                  # [B, NQ]
    refT = np.ascontiguousarray(ref.transpose(0, 2, 1))  # [B, D, NR]
    qT = np.ascontiguousarray(query.transpose(0, 2, 1))  # [B, D, NQ]

    nc = _build_program()

    iconrow = float(GRP - 1) - np.arange(GRP, dtype=np.float32)
    icon = np.broadcast_to(iconrow[None, :], (128, GRP)).copy()
    mlhs = np.ones((1, 128), dtype=np.float32)
    mrhs = np.full((1, 512), MAGIC, dtype=np.float32)
    nrhs = np.full((1, 512), -MAGIC, dtype=np.float32)
    ic0 = iconrow[None, :512].copy()
    ic1 = iconrow[None, 512:].copy()

    in_maps = []
    for core in range(NCORES):
        b, h = core // 2, core % 2
        lhs = np.empty((66, QPC), dtype=np.float32)
        lhs[0:D, :] = 65536.0 * qT[b][:, h * QPC:(h + 1) * QPC]
        lhs[D, :] = 32768.0
        lhs[D + 1, :] = 32768.0 * q2[b, h * QPC:(h + 1) * QPC]
        rhs = np.empty((66, NR), dtype=np.float32)
        rhs[0:D, :] = refT[b]
        rhs[D, :] = CCONST - r2[b]
        rhs[D + 1, :] = -1.0
        in_maps.append(
            {"lhs": lhs, "rhs": rhs, "icon": icon, "mlhs": mlhs,
             "mrhs": mrhs, "nrhs": nrhs, "ic0": ic0, "ic1": ic1}
        )

    res = run_bass_kernel_spmd(nc, in_maps, core_ids=list(range(NCORES)))

    Dout = np.empty((B, NQ, K), dtype=np.float32)
    Iout = np.empty((B, NQ, K), dtype=np.int64)
    for core in range(NCORES):
        b, h = core // 2, core % 2
        keys = res.results[core]["outK"]                  # [QPC, NCAND] f32
        kint = keys.astype(np.int64)                      # exact ints < 2^24
        top = np.argpartition(-keys, T32, axis=1)[:, :T32]
        kv = np.take_along_axis(kint, top, axis=1)
        j = (GRP - 1) - (kv & (GRP - 1))
        idx = (top >> 3) * GRP + j                        # [QPC, T32]

        # exact float64 rerank of the 32 candidates
        qb = query[b, h * QPC:(h + 1) * QPC].astype(np.float64)   # [QPC, D]
        rsel = ref[b].astype(np.float64)[idx]                     # [QPC, T32, D]
        diff = qb[:, None, :] - rsel
        d2 = np.einsum("qtd,qtd->qt", diff, diff)
        order = np.lexsort((idx, d2), axis=1)[:, :K]
        selI = np.take_along_axis(idx, order, axis=1)
        selD = np.sqrt(np.take_along_axis(d2, order, axis=1))

        Dout[b, h * QPC:(h + 1) * QPC] = selD.astype(np.float32)
        Iout[b, h * QPC:(h + 1) * QPC] = selI
    return (Dout, Iout)


# revision 11
# speedup vs baseline: 1.8138x; 1.0307x over previous
import sys

sys.path.insert(0, "/opt/trn_rl_repo")

import numpy as np

import concourse.bacc as bacc
import concourse.bass as bass
import concourse.mybir as mybir
import concourse.tile as tile
from concourse.bass_utils import run_bass_kernel_spmd

# Problem shapes (hardcoded per contract)
B = 4
NQ = 2048
NR = 16384
D = 64
K = 16

NCORES = 8
QPC = NQ // 2          # queries per core (each batch split across 2 cores)
NCHUNK = QPC // 128    # query chunks of 128 per core
GRP = 1024             # candidate group width (2 PSUM banks); top-8 per group
NGRP = NR // GRP       # 16 groups
NCAND = NGRP * 8       # 128 candidates per row

# Packed-key selection (one DVE Max8 sweep replaces Max+MaxIndex):
#   psum = 2^15*(C - d2)                  (scaled matmul, fp32)
#   += 2^33 ; -= 2^33 (1-row matmuls)     (fp32 accumulate rounds to m*1024)
#   key  = psum + (1023-j)                (exact ints < 2^24; j = idx in group)
# Max key == min d2 quantized to 1/32, ties -> lower idx. Host merges the
# top-32 of 128 candidate keys per row and reranks with exact float64 d2.
#
# The icon (1023-j) add is routed per group to spread engine load:
#   'pe'  : 4th accumulating 1-row matmul; Max8 reads the key from PSUM
#   'dve' : DVE STT key = (icon + 0) + psum -> SBUF
#   'pool': Act evicts v = Copy(psum - 2^33) (skips PE demagic), then
#           Pool TT key = v + icon
CCONST = 448.0
MAGIC = 2.0 ** 33
T32 = 32               # candidates reranked on host

ROUTES = [
    "dve", "pool", "pe", "pool", "pe", "pool", "pe", "pool",
    "pe", "pool", "pe", "pool", "pe", "pool", "pe", "pool",
]

_prog_cache = {}


def _build_program(reps: int = 1):
    if reps in _prog_cache:
        return _prog_cache[reps]

    f32 = mybir.dt.float32
    f32r = mybir.dt.float32r
    add = mybir.AluOpType.add
    Copy = mybir.ActivationFunctionType.Copy

    nc = bacc.Bacc("TRN2", target_bir_lowering=False, debug=False, num_devices=NCORES)

    # lhsT rows 0..63 = 2^16*q^T, row 64 = 2^15, row 65 = 2^15*q2
    # rhs  rows 0..63 = r^T,      row 64 = C-r2, row 65 = -1
    #   -> psum = 2^15 * (2q.r + C - r2 - q2) = 2^15 * (C - d2)
    lhs_d = nc.dram_tensor("lhs", [66, QPC], f32r, kind="ExternalInput")
    rhs_d = nc.dram_tensor("rhs", [66, NR], f32r, kind="ExternalInput")
    icon_d = nc.dram_tensor("icon", [128, GRP], f32, kind="ExternalInput")
    mlhs_d = nc.dram_tensor("mlhs", [1, 128], f32r, kind="ExternalInput")
    mrhs_d = nc.dram_tensor("mrhs", [1, 512], f32r, kind="ExternalInput")
    nrhs_d = nc.dram_tensor("nrhs", [1, 512], f32r, kind="ExternalInput")
    ic0_d = nc.dram_tensor("ic0", [1, 512], f32r, kind="ExternalInput")
    ic1_d = nc.dram_tensor("ic1", [1, 512], f32r, kind="ExternalInput")

    outK_d = nc.dram_tensor("outK", [QPC, NCAND], f32, kind="ExternalOutput")

    NRHS = 4  # rhs loaded as 4 tiles so compute starts before the full load
    RW = NR // NRHS

    with tile.TileContext(nc) as tc:
        with (
            tc.tile_pool(name="consts", bufs=1) as cpool,
            tc.tile_pool(name="psum", bufs=4, space="PSUM") as ppool,
            tc.tile_pool(name="keys", bufs=6) as kpool,
            tc.tile_pool(name="vtmp", bufs=4) as vpool,
            tc.tile_pool(name="cands", bufs=3) as candpool,
        ):
            mlhs_t = cpool.tile([1, 128], f32r, tag="mlhs")
            nc.sync.dma_start(mlhs_t[:], mlhs_d.ap())
            mrhs_t = cpool.tile([1, 512], f32r, tag="mrhs")
            nc.sync.dma_start(mrhs_t[:], mrhs_d.ap())
            nrhs_t = cpool.tile([1, 512], f32r, tag="nrhs")
            nc.sync.dma_start(nrhs_t[:], nrhs_d.ap())
            ic_t = []
            for nm, d in (("ic0", ic0_d), ("ic1", ic1_d)):
                t = cpool.tile([1, 512], f32r, tag=nm)
                nc.sync.dma_start(t[:], d.ap())
                ic_t.append(t)
            lhs_t = cpool.tile([66, QPC], f32r, tag="lhs")
            nc.sync.dma_start(lhs_t[:], lhs_d.ap())
            rhs0_t = cpool.tile([66, RW], f32r, tag="rhs0")
            nc.sync.dma_start(rhs0_t[:], rhs_d.ap()[:, 0:RW])
            icon_t = cpool.tile([128, GRP], f32, tag="icon")
            nc.sync.dma_start(icon_t[:], icon_d.ap())
            rhs_t = [rhs0_t]
            for i in range(1, NRHS):
                t = cpool.tile([66, RW], f32r, tag=f"rhs{i}")
                nc.sync.dma_start(t[:], rhs_d.ap()[:, i * RW:(i + 1) * RW])
                rhs_t.append(t)

            for rep in range(reps):
              for c in range(NCHUNK):
                cands = candpool.tile([128, NCAND], f32, tag="cv")
                for g in range(NGRP):
                    route = ROUTES[g]
                    rt = rhs_t[(g * GRP) // RW]
                    roff = (g * GRP) % RW
                    ps = ppool.tile([128, GRP], f32, tag="ps")
                    for h in range(2):
                        bank = ps[:, h * 512:(h + 1) * 512]
                        nc.tensor.matmul(
                            bank,
                            lhs_t[:, c * 128:(c + 1) * 128],
                            rt[:, roff + h * 512:roff + (h + 1) * 512],
                            start=True, stop=False,
                        )
                        # +2^33: rounds psum to the 1024 grid
                        last = route == "pool"
                        nc.tensor.matmul(bank, mlhs_t[:], mrhs_t[:],
                                         start=False, stop=last)
                        if not last:
                            # -2^33: psum = m*1024 exactly
                            nc.tensor.matmul(bank, mlhs_t[:], nrhs_t[:],
                                             start=False, stop=route != "pe")
                            if route == "pe":
                                # + (1023-j): key complete in PSUM
                                nc.tensor.matmul(bank, mlhs_t[:], ic_t[h][:],
                                                 start=False, stop=True)

                    s = g * 8
                    if route == "pe":
                        nc.vector.max(cands[:, s:s + 8], ps[:])
                        continue
                    key = kpool.tile([128, GRP], f32, tag="key")
                    if route == "dve":
                        nc.vector.scalar_tensor_tensor(
                            key[:], icon_t[:], 0.0, ps[:], op0=add, op1=add
                        )
                    else:  # pool
                        v = vpool.tile([128, GRP], f32, tag="v")
                        nc.scalar.activation(v[:], ps[:], Copy, bias=-MAGIC)
                        nc.gpsimd.tensor_tensor(
                            out=key[:], in0=v[:], in1=icon_t[:], op=add
                        )
                    nc.vector.max(cands[:, s:s + 8], key[:])

                r0, r1 = c * 128, (c + 1) * 128
                nc.sync.dma_start(outK_d.ap()[r0:r1, :], cands[:])

    nc.compile()
    _prog_cache[reps] = nc
    return nc


def kernel(ref: np.ndarray, query: np.ndarray):
    ref = np.asarray(ref, dtype=np.float32)
    query = np.asarray(query, dtype=np.float32)

    # host-side operand prep (layout + norms)
    r2 = np.sum(ref * ref, axis=-1)                      # [B, NR]
    q2 = np.sum(query * query, axis=-1)                  # [B, NQ]
    refT = np.ascontiguousarray(ref.transpose(0, 2, 1))  # [B, D, NR]
    qT = np.ascontiguousarray(query.transpose(0, 2, 1))  # [B, D, NQ]

    nc = _build_program()

    iconrow = float(GRP - 1) - np.arange(GRP, dtype=np.float32)
    icon = np.broadcast_to(iconrow[None, :], (128, GRP)).copy()
    mlhs = np.ones((1, 128), dtype=np.float32)
    mrhs = np.full((1, 512), MAGIC, dtype=np.float32)
    nrhs = np.full((1, 512), -MAGIC, dtype=np.float32)
    ic0 = iconrow[None, :512].copy()
    ic1 = iconrow[None, 512:].copy()

    in_maps = []
    for core in range(NCORES):
        b, h = core // 2, core % 2
        lhs = np.empty((66, QPC), dtype=np.float32)
        lhs[0:D, :] = 65536.0 * qT[b][:, h * QPC:(h + 1) * QPC]
        lhs[D, :] = 32768.0
        lhs[D + 1, :] = 32768.0 * q2[b, h * QPC:(h + 1) * QPC]
        rhs = np.empty((66, NR), dtype=np.float32)
        rhs[0:D, :] = refT[b]
        rhs[D, :] = CCONST - r2[b]
        rhs[D + 1, :] = -1.0
        in_maps.append(
            {"lhs": lhs, "rhs": rhs, "icon": icon, "mlhs": mlhs,
             "mrhs": mrhs, "nrhs": nrhs, "ic0": ic0, "ic1": ic1}
        )

    res = run_bass_kernel_spmd(nc, in_maps, core_ids=list(range(NCORES)))

    Dout = np.empty((B, NQ, K), dtype=np.float32)
    Iout = np.empty((B, NQ, K), dtype=np.int64)
    for core in range(NCORES):
        b, h = core // 2, core % 2
        keys = res.results[core]["outK"]                  # [QPC, NCAND] f32
        kint = keys.astype(np.int64)                      # exact ints < 2^24
        top = np.argpartition(-keys, T32, axis=1)[:, :T32]
        kv = np.take_along_axis(kint, top, axis=1)
        j = (GRP - 1) - (kv & (GRP - 1))
        idx = (top >> 3) * GRP + j                        # [QPC, T32]

        # exact float64 rerank of the 32 candidates
        qb = query[b, h * QPC:(h + 1) * QPC].astype(np.float64)   # [QPC, D]
        rsel = ref[b].astype(np.float64)[idx]                     # [QPC, T32, D]
        diff = qb[:, None, :] - rsel
        d2 = np.einsum("qtd,qtd->qt", diff, diff)
        order = np.lexsort((idx, d2), axis=1)[:, :K]
        selI = np.take_along_axis(idx, order, axis=1)
        selD = np.sqrt(np.take_along_axis(d2, order, axis=1))

        Dout[b, h * QPC:(h + 1) * QPC] = selD.astype(np.float32)
        Iout[b, h * QPC:(h + 1) * QPC] = selI
    return (Dout, Iout)
